# revision 7
# baseline (speedup 1.0000x reference)
"""Chamfer loss kernel for Trainium2 (8 NeuronCores, SPMD data-parallel over batch).

Full inputs x,y: [16, 2048, 4] f32. Output: scalar f32.

Math: dist[b,i,j] = ||x[b,i]-y[b,j]||; answer = sum_b (sum_i min_j dist + sum_j min_i dist).
d2[i,j] = |x_i|^2 + |y_j|^2 - 2 x_i.y_j is produced directly by one K=6 fp32 matmul
with augmented operands  lhsT = [-2x; |x|^2; 1] (6 x 128 chunk),  rhs = [y; 1; |y|^2] (6 x 512).
Per 128-row chunk the 4 matmuls fill one 4-bank PSUM stripe [128, 2048]; ScalarE converts
it to fp16 in SBUF; VectorE then computes the row-min (dir1) and a running elementwise
min across chunks (dir2).  dir2 finishes with sqrt -> PE transpose -> free-dim min.
Partition sum of the [128,1] accumulator via a ones-matmul. Host sums the 8 core scalars.
"""

import numpy as np
from contextlib import ExitStack

B, N, D = 16, 2048, 4
NCORES = 8
BPC = B // NCORES          # batches per core
NCH = N // 128             # 16 row chunks
K = 6                      # augmented contraction dim
F16_BIG = 60000.0

_CACHE: dict = {}


def _build_program(dve_dtype_name: str = "float16", reps: int = 1):
    import concourse.bacc as bacc
    import concourse.tile as tile
    from concourse import mybir

    f32 = mybir.dt.float32
    f16 = getattr(mybir.dt, dve_dtype_name)
    AX = mybir.AxisListType
    OP = mybir.AluOpType
    AF = mybir.ActivationFunctionType

    nc = bacc.Bacc("TRN2", target_bir_lowering=False, debug=False)
    xa_d = nc.dram_tensor("xa", [BPC, K, N], f32, kind="ExternalInput").ap()
    yb_d = nc.dram_tensor("yb", [BPC, K, N], f32, kind="ExternalInput").ap()
    aux_d = nc.dram_tensor("aux", [128, 129], f32, kind="ExternalInput").ap()
    out_d = nc.dram_tensor("out", [1, 1], f32, kind="ExternalOutput").ap()

    with ExitStack() as ctx:
        tc = ctx.enter_context(tile.TileContext(nc))
        iop = ctx.enter_context(tc.tile_pool(name="io", bufs=2))
        cst = ctx.enter_context(tc.tile_pool(name="cst", bufs=1))
        cpool = ctx.enter_context(tc.tile_pool(name="c", bufs=3))
        rpool = ctx.enter_context(tc.tile_pool(name="r", bufs=3))
        mpool = ctx.enter_context(tc.tile_pool(name="m", bufs=2))
        spool = ctx.enter_context(tc.tile_pool(name="s", bufs=2))
        apool = ctx.enter_context(tc.tile_pool(name="a", bufs=1))
        ps = ctx.enter_context(tc.tile_pool(name="ps", bufs=2, space="PSUM"))

        aux = cst.tile([128, 129], f32)
        nc.sync.dma_start(aux[:], aux_d[:])
        ident = aux[:, 0:128]
        ones = aux[:, 128:129]

        s_tot = apool.tile([128, 1], f32)
        nc.vector.memset(s_tot[:], 0.0)

        for b in [b for _ in range(reps) for b in range(BPC)]:
            xa = iop.tile([K, N], f32, tag="xa")
            nc.sync.dma_start(xa[:], xa_d[b])
            yb = iop.tile([K, N], f32, tag="yb")
            nc.sync.dma_start(yb[:], yb_d[b])

            m = mpool.tile([128, NCH], f16, tag="m")
            R = None
            for k in range(NCH):
                pt = ps.tile([128, N], f32, tag="mm")
                for j in range(N // 512):
                    nc.tensor.matmul(
                        pt[:, j * 512:(j + 1) * 512],
                        xa[:, k * 128:(k + 1) * 128],
                        yb[:, j * 512:(j + 1) * 512],
                        start=True, stop=True,
                    )
                c = cpool.tile([128, N], f16, tag="c")
                nc.scalar.activation(c[:], pt[:], AF.Relu)
                # dir2 running elementwise min across row chunks
                Rn = rpool.tile([128, N], f16, tag="R")
                if k == 0:
                    nc.vector.tensor_copy(Rn[:], c[:])
                else:
                    nc.vector.tensor_tensor(Rn[:], c[:], R[:], op=OP.min)
                R = Rn
                # dir1 row-min: fold 2048 -> 1024 -> 512, then reduce
                nc.vector.tensor_tensor(c[:, 0:1024], c[:, 0:1024], c[:, 1024:2048], op=OP.min)
                nc.vector.tensor_tensor(c[:, 0:512], c[:, 0:512], c[:, 512:1024], op=OP.min)
                nc.vector.tensor_reduce(m[:, k:k + 1], c[:, 0:512], axis=AX.X, op=OP.min)

            # dir1: dist = sqrt(min d2); accumulate sum over the 16 chunk-mins
            sq1 = spool.tile([128, NCH], f32, tag="sq1")
            s1 = spool.tile([128, 1], f32, tag="s1")
            nc.scalar.activation(sq1[:], m[:], AF.Sqrt, accum_out=s1[:])
            nc.vector.tensor_add(s_tot[:], s_tot[:], s1[:])

            # dir2 tail: sqrt first (monotone), transpose 16 chunks, min over free dim
            S = spool.tile([128, N], f32, tag="S")
            nc.scalar.activation(S[:], R[:], AF.Sqrt)
            tp = ps.tile([128, N], f32, tag="mm")
            for k in range(NCH):
                nc.tensor.transpose(tp[:, k * 128:(k + 1) * 128], S[:, k * 128:(k + 1) * 128], ident)
            m2 = spool.tile([128, NCH], f32, tag="m2")
            nc.vector.tensor_reduce(m2[:], tp[:].rearrange("p (c f) -> p c f", c=NCH), axis=AX.X, op=OP.min)
            s2 = spool.tile([128, 1], f32, tag="s2")
            nc.vector.tensor_reduce(s2[:], m2[:], axis=AX.X, op=OP.add)
            nc.vector.tensor_add(s_tot[:], s_tot[:], s2[:])

        pfin = ps.tile([1, 1], f32, tag="mm")
        nc.tensor.matmul(pfin[:], s_tot[:], ones, start=True, stop=True)
        ob = spool.tile([1, 1], f32, tag="ob")
        nc.scalar.copy(ob[:], pfin[:])
        nc.sync.dma_start(out_d[:], ob[:])

    nc.compile()
    return nc


def _get_nc():
    if "nc" not in _CACHE:
        _CACHE["nc"] = _build_program()
    return _CACHE["nc"]


def make_in_maps(x: np.ndarray, y: np.ndarray):
    x = np.ascontiguousarray(np.asarray(x, dtype=np.float32))
    y = np.ascontiguousarray(np.asarray(y, dtype=np.float32))
    xsq = np.einsum("bnd,bnd->bn", x, x)
    ysq = np.einsum("bnd,bnd->bn", y, y)
    XA = np.empty((B, K, N), np.float32)
    XA[:, 0:D] = -2.0 * x.transpose(0, 2, 1)
    XA[:, D] = xsq
    XA[:, D + 1] = 1.0
    YB = np.empty((B, K, N), np.float32)
    YB[:, 0:D] = y.transpose(0, 2, 1)
    YB[:, D] = 1.0
    YB[:, D + 1] = ysq
    aux = np.zeros((128, 129), np.float32)
    aux[:, :128] = np.eye(128, dtype=np.float32)
    aux[:, 128] = 1.0
    return [
        {
            "xa": np.ascontiguousarray(XA[BPC * c:BPC * (c + 1)]),
            "yb": np.ascontiguousarray(YB[BPC * c:BPC * (c + 1)]),
            "aux": aux,
        }
        for c in range(NCORES)
    ]


def kernel(x: np.ndarray, y: np.ndarray) -> np.ndarray:
    from concourse.bass_utils import run_bass_kernel_spmd

    in_maps = make_in_maps(x, y)
    nc = _get_nc()
    res = run_bass_kernel_spmd(nc, in_maps, core_ids=list(range(NCORES)))
    total = np.float64(0.0)
    for r in res.results:
        total += np.float64(r["out"][0, 0])
    return np.asarray(total, dtype=np.float32)
